# revision 3
# baseline (speedup 1.0000x reference)
"""BitLinear on 8 trn2 cores — data-parallel rows + cooperative ternary weight.

Sharding: core c owns x rows [c*1024, (c+1)*1024); its w_in carries THREE
o-shards: rows of cores c, c+1, c+2 (mod 8), 512 each. Each core:
  1. abs-sums shard 0 (its own) -> tiny AllGather + local sum -> global
     absmean scale. The pass-1 loads go FIRST so the collective's input
     is not delayed by HBM contention (the mesh algo itself is ~30us).
  2. loads + int8-fake-quantizes its 8 x row-tiles (SBUF-resident xqT)
  3. ternarizes its three local shards into wqT chunks [i, o] (bf16);
     chunk 0 is cast-DMA'd to int8 DRAM and AllGathered (16.8MB);
     chunks 1-2 are consumed straight from SBUF, hiding the collective
  4. matmuls chunk-by-chunk over o: ci=0,1,2 local, ci=3..7 from the
     gather; o-column slots are addressed with the partition id so the
     SPMD program is core-agnostic.

Scheduling invariants learned the hard way:
  - XBAR transposes serialize against in-flight collectives, so every
    transpose is ordered before the exchange trigger (dep edges), and
    the scale collective completes (~60us) before the first transpose.
  - Engines are in-order: loads are fire-and-forget triggers, but a
    waiting op at a queue head blocks everything behind it. All loads
    go on sync in priority order; compute queues carry only compute.
  - The wt pool is allocated statically (never reuses scoped x-pool
    space) so the first w transpose never waits for the x drain.

The quantized operands are exact small integers in bf16; accumulation in
fp32 PSUM is exact; PSUM is scaled by s_row*s_w on eviction (identical
numerics to the reference up to matmul associativity).
"""

from contextlib import ExitStack

import numpy as np

import concourse.mybir as mybir
import concourse.tile as tile
from concourse import bacc, bass_isa
from concourse.bass import ts
from concourse.bass_utils import run_bass_kernel_spmd
from concourse.tile_rust import add_dep_helper

F32 = mybir.dt.float32
BF16 = mybir.dt.bfloat16
I8 = mybir.dt.int8

MAGIC = 12582912.0  # 1.5 * 2^23: fp32 RNE rounder for |v| < 2^22
N_CORES = 8
P = 128
IN_F = 4096                   # contraction dim (i)
OUT_F = 4096
K_TILES = IN_F // P           # 32
W_SHARD = OUT_F // N_CORES    # 512 o-rows of weight per chunk
W_TILES = W_SHARD // P        # 4
N_LOC = 3                     # local chunks per core (own + 2 neighbors)
M_LOC = 8                     # x row-tiles per core (full problem)

# f32-exact constants mirroring the reference arithmetic
_MEAN_C = float(np.float32(2.0**-24))                    # 1/(4096*4096)
_EPS = float(np.float32(1e-8))
_SW127_C = float(np.float32(np.float32(2.0**-24) * np.float32(1.0 / 127.0)))


def _body(ctx, tc, x_ap, w_ap, wql_ap, wqa_ap, o_ap, m_loc):
    nc = tc.nc
    pid = nc.gpsimd.partition_id()
    slots = [(pid + ci) % N_CORES for ci in range(N_CORES)]

    const = ctx.enter_context(tc.tile_pool(name="const", bufs=1))
    dramp = ctx.enter_context(tc.tile_pool(name="dram", bufs=1, space="DRAM"))
    xqtp = ctx.enter_context(tc.tile_pool(name="xqt", bufs=m_loc))
    wp = ctx.enter_context(tc.tile_pool(name="w", bufs=2))
    xqp = ctx.enter_context(tc.tile_pool(name="xq", bufs=1))
    wtp = ctx.enter_context(tc.tile_pool(name="wt", bufs=2))
    psump = ctx.enter_context(tc.tile_pool(name="psum", bufs=6, space="PSUM"))
    statp = ctx.enter_context(tc.tile_pool(name="stat", bufs=2))

    xctx = ExitStack()  # x fp32 staging, closed before wq/out pools open
    xp = xctx.enter_context(tc.tile_pool(name="x", bufs=2))

    # ---- pass 1 FIRST: w shard-0 load (sync) + abs partial sums (DVE)
    partials = const.tile([P, W_TILES], F32)
    w1s = []
    for t in range(W_TILES):
        wt = wp.tile([P, IN_F], F32, tag="wl")
        nc.sync.dma_start(wt[:], w_ap[t * P:(t + 1) * P, :])
        w1s.append(wt)
    for t in range(W_TILES):
        nc.vector.tensor_reduce(partials[:, t:t + 1], w1s[t][:],
                                axis=mybir.AxisListType.X,
                                op=mybir.AluOpType.add,
                                apply_absolute_value=True)
    del w1s
    p1 = const.tile([P, 1], F32)
    nc.vector.tensor_reduce(p1[:], partials[:], axis=mybir.AxisListType.X,
                            op=mybir.AluOpType.add)

    # x loads right behind the pass-1 loads on sync
    xs = {}
    for mt in range(m_loc):
        x = xp.tile([P, IN_F], F32, tag="x", name=f"x{mt}")
        nc.sync.dma_start(x[:], x_ap[mt * P:(mt + 1) * P, :])
        xs[mt] = x

    # ---- tiny AllGather of the per-core |w| partial sums (gpsimd queue)
    pa = const.tile([P, 1], F32)
    nc.gpsimd.partition_all_reduce(pa[:], p1[:], channels=P,
                                   reduce_op=bass_isa.ReduceOp.add)
    cc_in = dramp.tile([1, 1], F32)
    cc_out = dramp.tile([N_CORES, 1], F32)
    nc.gpsimd.dma_start(cc_in[:], pa[:1, :1])
    nc.gpsimd.collective_compute(
        "AllGather", mybir.AluOpType.bypass,
        replica_groups=[list(range(N_CORES))],
        ins=[cc_in[:].opt()], outs=[cc_out[:].opt()],
    )

    # ---- x quantize chain; transposes are xbar (the collective is done
    # by the time xq tiles exist) and pace the chain via xq bufs=1
    mrows = {}
    stots = {}
    xqts = {}

    def x_stats(mt):
        x = xs[mt]
        mrow = statp.tile([P, 1], F32, tag=f"mrow{mt}", bufs=1)
        nc.vector.tensor_reduce(mrow[:], x[:], axis=mybir.AxisListType.X,
                                op=mybir.AluOpType.max,
                                apply_absolute_value=True)
        r127 = statp.tile([P, 1], F32, tag="r127")
        nc.vector.reciprocal(r127[:], mrow[:])
        nc.vector.tensor_scalar_mul(r127[:], r127[:], 127.0)
        nc.scalar.activation(x[:], x[:], mybir.ActivationFunctionType.Copy,
                             bias=MAGIC, scale=r127[:])
        mrows[mt] = mrow

    def x_finish(mt):
        x = xs.pop(mt)
        xq = xqp.tile([P, IN_F], BF16, tag="xq")
        nc.vector.tensor_scalar_sub(xq[:], x[:], MAGIC)
        xqT = xqtp.tile([P, K_TILES, P], BF16, tag="xqT")
        nc.sync.dma_start_transpose(xqT[:], xq[:])
        xqts[mt] = xqT

    for mt in range(m_loc):
        x_stats(mt)
        if mt >= 1:
            x_finish(mt - 1)
    x_finish(m_loc - 1)

    # pass-2 prefetch for the first wp buffers (sync, behind x loads)
    n_wq = N_LOC * W_TILES
    wt2s = []

    def w2_load(t):
        # gpsimd, not sync: on sync these triggers interleave between the
        # chained x transposes and stall every w-quant round behind them
        wt = wp.tile([P, IN_F], F32, tag="wl", name=f"w2_{t}")
        nc.gpsimd.dma_start(wt[:], w_ap[t * P:(t + 1) * P, :])
        wt2s.append(wt)

    for t in range(min(2, n_wq)):
        w2_load(t)
    xctx.close()

    wqp = ctx.enter_context(tc.tile_pool(name="wq", bufs=2))
    outp = ctx.enter_context(tc.tile_pool(name="out", bufs=4))

    # ---- collective result -> scales
    gs8 = const.tile([1, N_CORES], F32)
    nc.gpsimd.dma_start(gs8[:], cc_out[:].rearrange("a b -> b a"))
    gs1 = const.tile([1, 1], F32)
    nc.vector.tensor_reduce(gs1[:], gs8[:], axis=mybir.AxisListType.X,
                            op=mybir.AluOpType.add)
    gsum = const.tile([P, 1], F32)
    nc.gpsimd.partition_broadcast(gsum[:], gs1[:])

    scale_eps = const.tile([P, 1], F32)
    nc.vector.tensor_scalar(scale_eps[:], gsum[:], _MEAN_C, _EPS,
                            op0=mybir.AluOpType.mult, op1=mybir.AluOpType.add)
    rec_w = const.tile([P, 1], F32)
    nc.vector.reciprocal(rec_w[:], scale_eps[:])
    sw127 = const.tile([P, 1], F32)
    nc.vector.tensor_scalar_mul(sw127[:], gsum[:], _SW127_C)
    for mt in range(m_loc):
        s_tot = statp.tile([P, 1], F32, tag=f"stot{mt}", bufs=1)
        nc.vector.tensor_tensor(s_tot[:], mrows[mt][:], sw127[:],
                                op=mybir.AluOpType.mult)
        stots[mt] = s_tot

    # ---- w pass 2: ternarize + transpose the three local chunks
    wqTs = {}
    t_insts = []

    def w_quant_chunk(g):
        wqT = wtp.tile([P, K_TILES, W_SHARD], BF16, tag="wT")
        for tt in range(W_TILES):
            t = g * W_TILES + tt
            if t + 2 < n_wq:
                w2_load(t + 2)
            wt = wt2s[t]
            wq = wqp.tile([P, IN_F], BF16, tag="wq")
            if t % 2 == 0:
                # ACT round, DVE (unmagic,min), DVE (max -> bf16)
                nc.scalar.activation(wt[:], wt[:],
                                     mybir.ActivationFunctionType.Copy,
                                     bias=MAGIC, scale=rec_w[:])
                nc.vector.tensor_scalar(wt[:], wt[:], MAGIC, 1.0,
                                        op0=mybir.AluOpType.subtract,
                                        op1=mybir.AluOpType.min)
                nc.vector.tensor_scalar_max(wq[:], wt[:], -1.0)
            else:
                # ACT round, ACT unmagic, DVE fused clip -> bf16
                nc.scalar.activation(wt[:], wt[:],
                                     mybir.ActivationFunctionType.Copy,
                                     bias=MAGIC, scale=rec_w[:])
                nc.scalar.activation(wt[:], wt[:],
                                     mybir.ActivationFunctionType.Copy,
                                     bias=-MAGIC)
                nc.vector.tensor_scalar(wq[:], wt[:], 1.0, -1.0,
                                        op0=mybir.AluOpType.min,
                                        op1=mybir.AluOpType.max)
            ti = nc.sync.dma_start_transpose(
                wqT[:, :, tt * P:(tt + 1) * P], wq[:])
            t_insts.append(ti)
        return wqT

    wqTs[0] = w_quant_chunk(0)
    # cast-DMA own chunk to int8 for the exchange
    nc.gpsimd.dma_start(wql_ap.rearrange("p (k o) -> p k o", k=K_TILES),
                        wqTs[0][:])
    for g in range(1, min(N_LOC, N_CORES)):
        wqTs[g] = w_quant_chunk(g)
    cc2 = nc.gpsimd.collective_compute(
        "AllGather", mybir.AluOpType.bypass,
        replica_groups=[list(range(N_CORES))],
        ins=[wql_ap.opt()], outs=[wqa_ap.opt()],
    )
    # xbar transposes serialize against in-flight collectives, so the
    # exchange must not start until every transpose has drained
    for ti in t_insts:
        add_dep_helper(cc2.ins, ti.ins,
                       reason="AllGather after all xbar transposes")

    # ---- main loop: chunks of 512 o-columns, local chunks first
    psums = {}
    ots = {}

    def load_chunk(ci):
        wT = wtp.tile([P, K_TILES, W_SHARD], BF16, tag="wT")
        nc.gpsimd.dma_start(
            wT[:],
            wqa_ap[ts(slots[ci], P), :]
            .rearrange("p (k o) -> p k o", k=K_TILES))
        return wT

    def mms(mt, wT, ci):
        ps = psump.tile([P, W_SHARD], F32, tag="ps")
        for k in range(K_TILES):
            nc.tensor.matmul(ps[:], xqts[mt][:, k, :], wT[:, k, :],
                             start=(k == 0), stop=(k == K_TILES - 1))
        psums[(mt, ci)] = ps

    def evict(mt, ci):
        ps = psums.pop((mt, ci))
        ot = outp.tile([P, W_SHARD], F32, tag="ot")
        nc.vector.tensor_scalar_mul(ot[:], ps[:], stots[mt][:])
        ots[(mt, ci)] = ot

    def store(mt, ci):
        nc.gpsimd.dma_start(
            o_ap[mt * P:(mt + 1) * P, ts(slots[ci], W_SHARD)],
            ots.pop((mt, ci)))

    wTs = dict(wqTs)
    done = []
    for ci in range(N_CORES):
        wT = wTs.pop(ci)
        for mt in range(m_loc):
            if mt == 1 and N_LOC <= ci + 1 < N_CORES:
                wTs[ci + 1] = load_chunk(ci + 1)
            mms(mt, wT, ci)
            done.append((mt, ci))
            if len(done) >= 2:
                evict(*done[-2])
            if len(done) >= 3:
                store(*done[-3])
    evict(*done[-1])
    for d in done[-2:]:
        store(*d)


_NC_CACHE = {}


def build_nc(m_loc):
    if m_loc in _NC_CACHE:
        return _NC_CACHE[m_loc]
    nc = bacc.Bacc("TRN2", target_bir_lowering=False, debug=False,
                   num_devices=N_CORES)
    rows = m_loc * P
    x_dram = nc.dram_tensor("x_in", [rows, IN_F], F32, kind="ExternalInput")
    w_dram = nc.dram_tensor("w_in", [N_LOC * W_SHARD, IN_F], F32,
                            kind="ExternalInput")
    wql = nc.dram_tensor("wq_loc", [P, K_TILES * W_SHARD], I8,
                         kind="Internal")
    wqa = nc.dram_tensor("wq_all", [N_CORES * P, K_TILES * W_SHARD], I8,
                         kind="Internal")
    o_dram = nc.dram_tensor("out", [rows, OUT_F], F32,
                            kind="ExternalOutput")
    with tile.TileContext(nc) as tc, ExitStack() as ctx:
        _body(ctx, tc, x_dram.ap(), w_dram.ap(), wql.ap(), wqa.ap(),
              o_dram.ap(), m_loc)
    nc.compile()
    _NC_CACHE[m_loc] = nc
    return nc


def run_sharded(x2d, weight, m_loc=M_LOC, trace=False):
    """x2d: [8192, 4096] f32 (full rows), weight: [4096, 4096] f32."""
    nc = build_nc(m_loc)
    rl = m_loc * P
    in_maps = []
    for c in range(N_CORES):
        wrows = [weight[((c + g) % N_CORES) * W_SHARD:
                        (((c + g) % N_CORES) + 1) * W_SHARD]
                 for g in range(N_LOC)]
        in_maps.append(
            {"x_in": np.ascontiguousarray(x2d[c * 1024:c * 1024 + rl]),
             "w_in": np.ascontiguousarray(np.concatenate(wrows, axis=0))})
    res = run_bass_kernel_spmd(nc, in_maps, core_ids=list(range(N_CORES)),
                               trace=trace)
    out = np.concatenate([res.results[c]["out"] for c in range(N_CORES)],
                         axis=0)
    rows = [c * 1024 + r for c in range(N_CORES) for r in range(rl)]
    return out, rows, res


def kernel(x, weight):
    b, s, f = x.shape
    x2d = np.ascontiguousarray(x.reshape(b * s, f)).astype(np.float32,
                                                           copy=False)
    w = np.ascontiguousarray(weight).astype(np.float32, copy=False)
    out, _, _ = run_sharded(x2d, w, M_LOC)
    return out.reshape(b, s, OUT_F).astype(np.float32, copy=False)


# revision 4
# speedup vs baseline: 1.0286x; 1.0286x over previous
"""BitLinear on 8 trn2 cores — data-parallel rows + cooperative ternary weight.

Sharding: core c owns x rows [c*1024, (c+1)*1024); its w_in carries THREE
o-shards: rows of cores c, c+1, c+2 (mod 8), 512 each. Each core:
  1. abs-sums shard 0 (its own) -> tiny AllGather + local sum -> global
     absmean scale. The pass-1 loads go FIRST so the collective's input
     is not delayed by HBM contention (the mesh algo itself is ~30us).
  2. loads + int8-fake-quantizes its 8 x row-tiles (SBUF-resident xqT)
  3. ternarizes its three local shards into wqT chunks [i, o] (bf16);
     chunk 0 is cast-DMA'd to int8 DRAM and AllGathered (16.8MB);
     chunks 1-2 are consumed straight from SBUF, hiding the collective
  4. matmuls chunk-by-chunk over o: ci=0,1,2 local, ci=3..7 from the
     gather; o-column slots are addressed with the partition id so the
     SPMD program is core-agnostic.

Scheduling invariants learned the hard way:
  - XBAR transposes serialize against in-flight collectives, so every
    transpose is ordered before the exchange trigger (dep edges), and
    the scale collective completes (~60us) before the first transpose.
  - Engines are in-order: loads are fire-and-forget triggers, but a
    waiting op at a queue head blocks everything behind it. All loads
    go on sync in priority order; compute queues carry only compute.
  - The wt pool is allocated statically (never reuses scoped x-pool
    space) so the first w transpose never waits for the x drain.

The quantized operands are exact small integers in bf16; accumulation in
fp32 PSUM is exact; PSUM is scaled by s_row*s_w on eviction (identical
numerics to the reference up to matmul associativity).
"""

from contextlib import ExitStack

import numpy as np

import concourse.mybir as mybir
import concourse.tile as tile
from concourse import bacc, bass_isa
from concourse.bass import ts
from concourse.bass_utils import run_bass_kernel_spmd
from concourse.tile_rust import add_dep_helper

F32 = mybir.dt.float32
BF16 = mybir.dt.bfloat16
I8 = mybir.dt.int8

MAGIC = 12582912.0  # 1.5 * 2^23: fp32 RNE rounder for |v| < 2^22
N_CORES = 8
P = 128
IN_F = 4096                   # contraction dim (i)
OUT_F = 4096
K_TILES = IN_F // P           # 32
W_SHARD = OUT_F // N_CORES    # 512 o-rows of weight per chunk
W_TILES = W_SHARD // P        # 4
N_LOC = 3                     # local chunks per core (own + 2 neighbors)
M_LOC = 8                     # x row-tiles per core (full problem)

# f32-exact constants mirroring the reference arithmetic
_MEAN_C = float(np.float32(2.0**-24))                    # 1/(4096*4096)
_EPS = float(np.float32(1e-8))
_SW127_C = float(np.float32(np.float32(2.0**-24) * np.float32(1.0 / 127.0)))


def _body(ctx, tc, x_ap, w_ap, wql_ap, wqa_ap, o_ap, m_loc):
    nc = tc.nc
    pid = nc.gpsimd.partition_id()
    slots = [(pid + ci) % N_CORES for ci in range(N_CORES)]

    const = ctx.enter_context(tc.tile_pool(name="const", bufs=1))
    dramp = ctx.enter_context(tc.tile_pool(name="dram", bufs=1, space="DRAM"))
    xqtp = ctx.enter_context(tc.tile_pool(name="xqt", bufs=m_loc))
    wp = ctx.enter_context(tc.tile_pool(name="w", bufs=2))
    xqp = ctx.enter_context(tc.tile_pool(name="xq", bufs=1))
    wtp = ctx.enter_context(tc.tile_pool(name="wt", bufs=2))
    psump = ctx.enter_context(tc.tile_pool(name="psum", bufs=6, space="PSUM"))
    statp = ctx.enter_context(tc.tile_pool(name="stat", bufs=2))

    xctx = ExitStack()  # x fp32 staging, closed before wq/out pools open
    xp = xctx.enter_context(tc.tile_pool(name="x", bufs=2))

    # ---- pass 1 FIRST: w shard-0 load (sync) + abs partial sums (DVE)
    partials = const.tile([P, W_TILES], F32)
    w1s = []
    for t in range(W_TILES):
        wt = wp.tile([P, IN_F], F32, tag="wl")
        nc.sync.dma_start(wt[:], w_ap[t * P:(t + 1) * P, :])
        w1s.append(wt)
    for t in range(W_TILES):
        nc.vector.tensor_reduce(partials[:, t:t + 1], w1s[t][:],
                                axis=mybir.AxisListType.X,
                                op=mybir.AluOpType.add,
                                apply_absolute_value=True)
    del w1s
    p1 = const.tile([P, 1], F32)
    nc.vector.tensor_reduce(p1[:], partials[:], axis=mybir.AxisListType.X,
                            op=mybir.AluOpType.add)

    # x loads right behind the pass-1 loads on sync
    xs = {}
    for mt in range(m_loc):
        x = xp.tile([P, IN_F], F32, tag="x", name=f"x{mt}")
        nc.sync.dma_start(x[:], x_ap[mt * P:(mt + 1) * P, :])
        xs[mt] = x

    # ---- tiny AllGather of the per-core |w| partial sums (gpsimd queue)
    pa = const.tile([P, 1], F32)
    nc.gpsimd.partition_all_reduce(pa[:], p1[:], channels=P,
                                   reduce_op=bass_isa.ReduceOp.add)
    cc_in = dramp.tile([1, 1], F32)
    cc_out = dramp.tile([N_CORES, 1], F32)
    nc.gpsimd.dma_start(cc_in[:], pa[:1, :1])
    nc.gpsimd.collective_compute(
        "AllGather", mybir.AluOpType.bypass,
        replica_groups=[list(range(N_CORES))],
        ins=[cc_in[:].opt()], outs=[cc_out[:].opt()],
    )

    # ---- x quantize chain; transposes are xbar (the collective is done
    # by the time xq tiles exist) and pace the chain via xq bufs=1
    mrows = {}
    stots = {}
    xqts = {}

    def x_stats(mt):
        x = xs[mt]
        mrow = statp.tile([P, 1], F32, tag=f"mrow{mt}", bufs=1)
        nc.vector.tensor_reduce(mrow[:], x[:], axis=mybir.AxisListType.X,
                                op=mybir.AluOpType.max,
                                apply_absolute_value=True)
        r127 = statp.tile([P, 1], F32, tag="r127")
        nc.vector.reciprocal(r127[:], mrow[:])
        nc.vector.tensor_scalar_mul(r127[:], r127[:], 127.0)
        nc.scalar.activation(x[:], x[:], mybir.ActivationFunctionType.Copy,
                             bias=MAGIC, scale=r127[:])
        mrows[mt] = mrow

    def x_finish(mt):
        x = xs.pop(mt)
        xq = xqp.tile([P, IN_F], BF16, tag="xq")
        nc.vector.tensor_scalar_sub(xq[:], x[:], MAGIC)
        xqT = xqtp.tile([P, K_TILES, P], BF16, tag="xqT")
        nc.sync.dma_start_transpose(xqT[:], xq[:])
        xqts[mt] = xqT

    for mt in range(m_loc):
        x_stats(mt)
        if mt >= 1:
            x_finish(mt - 1)
    x_finish(m_loc - 1)

    # pass-2 prefetch for the first wp buffers (sync, behind x loads)
    n_wq = N_LOC * W_TILES
    wt2s = []

    def w2_load(t):
        wt = wp.tile([P, IN_F], F32, tag="wl", name=f"w2_{t}")
        nc.sync.dma_start(wt[:], w_ap[t * P:(t + 1) * P, :])
        wt2s.append(wt)

    for t in range(min(2, n_wq)):
        w2_load(t)
    xctx.close()

    wqp = ctx.enter_context(tc.tile_pool(name="wq", bufs=2))
    outp = ctx.enter_context(tc.tile_pool(name="out", bufs=4))

    # ---- collective result -> scales
    gs8 = const.tile([1, N_CORES], F32)
    nc.gpsimd.dma_start(gs8[:], cc_out[:].rearrange("a b -> b a"))
    gs1 = const.tile([1, 1], F32)
    nc.vector.tensor_reduce(gs1[:], gs8[:], axis=mybir.AxisListType.X,
                            op=mybir.AluOpType.add)
    gsum = const.tile([P, 1], F32)
    nc.gpsimd.partition_broadcast(gsum[:], gs1[:])

    scale_eps = const.tile([P, 1], F32)
    nc.vector.tensor_scalar(scale_eps[:], gsum[:], _MEAN_C, _EPS,
                            op0=mybir.AluOpType.mult, op1=mybir.AluOpType.add)
    rec_w = const.tile([P, 1], F32)
    nc.vector.reciprocal(rec_w[:], scale_eps[:])
    sw127 = const.tile([P, 1], F32)
    nc.vector.tensor_scalar_mul(sw127[:], gsum[:], _SW127_C)
    for mt in range(m_loc):
        s_tot = statp.tile([P, 1], F32, tag=f"stot{mt}", bufs=1)
        nc.vector.tensor_tensor(s_tot[:], mrows[mt][:], sw127[:],
                                op=mybir.AluOpType.mult)
        stots[mt] = s_tot

    # ---- w pass 2: ternarize + transpose the three local chunks
    wqTs = {}
    t_insts = []

    def w_quant_chunk(g):
        wqT = wtp.tile([P, K_TILES, W_SHARD], BF16, tag="wT")
        for tt in range(W_TILES):
            t = g * W_TILES + tt
            if t + 2 < n_wq:
                w2_load(t + 2)
            wt = wt2s[t]
            wq = wqp.tile([P, IN_F], BF16, tag="wq")
            if t % 2 == 0:
                # ACT round, DVE (unmagic,min), DVE (max -> bf16)
                nc.scalar.activation(wt[:], wt[:],
                                     mybir.ActivationFunctionType.Copy,
                                     bias=MAGIC, scale=rec_w[:])
                nc.vector.tensor_scalar(wt[:], wt[:], MAGIC, 1.0,
                                        op0=mybir.AluOpType.subtract,
                                        op1=mybir.AluOpType.min)
                nc.vector.tensor_scalar_max(wq[:], wt[:], -1.0)
            else:
                # ACT round, ACT unmagic, DVE fused clip -> bf16
                nc.scalar.activation(wt[:], wt[:],
                                     mybir.ActivationFunctionType.Copy,
                                     bias=MAGIC, scale=rec_w[:])
                nc.scalar.activation(wt[:], wt[:],
                                     mybir.ActivationFunctionType.Copy,
                                     bias=-MAGIC)
                nc.vector.tensor_scalar(wq[:], wt[:], 1.0, -1.0,
                                        op0=mybir.AluOpType.min,
                                        op1=mybir.AluOpType.max)
            ti = nc.sync.dma_start_transpose(
                wqT[:, :, tt * P:(tt + 1) * P], wq[:])
            t_insts.append(ti)
        return wqT

    wqTs[0] = w_quant_chunk(0)
    # cast-DMA own chunk to int8 for the exchange
    nc.gpsimd.dma_start(wql_ap.rearrange("p (k o) -> p k o", k=K_TILES),
                        wqTs[0][:])
    for g in range(1, min(N_LOC, N_CORES)):
        wqTs[g] = w_quant_chunk(g)
    cc2 = nc.gpsimd.collective_compute(
        "AllGather", mybir.AluOpType.bypass,
        replica_groups=[list(range(N_CORES))],
        ins=[wql_ap.opt()], outs=[wqa_ap.opt()],
    )
    # xbar transposes serialize against in-flight collectives, so the
    # exchange must not start until every transpose has drained
    for ti in t_insts:
        add_dep_helper(cc2.ins, ti.ins,
                       reason="AllGather after all xbar transposes")

    # ---- main loop: chunks of 512 o-columns, local chunks first
    psums = {}
    ots = {}

    def load_chunk(ci):
        wT = wtp.tile([P, K_TILES, W_SHARD], BF16, tag="wT")
        nc.gpsimd.dma_start(
            wT[:],
            wqa_ap[ts(slots[ci], P), :]
            .rearrange("p (k o) -> p k o", k=K_TILES))
        return wT

    def mms(mt, wT, ci):
        ps = psump.tile([P, W_SHARD], F32, tag="ps")
        for k in range(K_TILES):
            nc.tensor.matmul(ps[:], xqts[mt][:, k, :], wT[:, k, :],
                             start=(k == 0), stop=(k == K_TILES - 1))
        psums[(mt, ci)] = ps

    def evict(mt, ci):
        ps = psums.pop((mt, ci))
        ot = outp.tile([P, W_SHARD], F32, tag="ot")
        nc.vector.tensor_scalar_mul(ot[:], ps[:], stots[mt][:])
        ots[(mt, ci)] = ot

    def store(mt, ci):
        nc.gpsimd.dma_start(
            o_ap[mt * P:(mt + 1) * P, ts(slots[ci], W_SHARD)],
            ots.pop((mt, ci)))

    wTs = dict(wqTs)
    done = []
    for ci in range(N_CORES):
        wT = wTs.pop(ci)
        for mt in range(m_loc):
            if mt == 1 and N_LOC <= ci + 1 < N_CORES:
                wTs[ci + 1] = load_chunk(ci + 1)
            mms(mt, wT, ci)
            done.append((mt, ci))
            if len(done) >= 2:
                evict(*done[-2])
            if len(done) >= 3:
                store(*done[-3])
    evict(*done[-1])
    for d in done[-2:]:
        store(*d)


_NC_CACHE = {}


def build_nc(m_loc):
    if m_loc in _NC_CACHE:
        return _NC_CACHE[m_loc]
    nc = bacc.Bacc("TRN2", target_bir_lowering=False, debug=False,
                   num_devices=N_CORES)
    rows = m_loc * P
    x_dram = nc.dram_tensor("x_in", [rows, IN_F], F32, kind="ExternalInput")
    w_dram = nc.dram_tensor("w_in", [N_LOC * W_SHARD, IN_F], F32,
                            kind="ExternalInput")
    wql = nc.dram_tensor("wq_loc", [P, K_TILES * W_SHARD], I8,
                         kind="Internal")
    wqa = nc.dram_tensor("wq_all", [N_CORES * P, K_TILES * W_SHARD], I8,
                         kind="Internal")
    o_dram = nc.dram_tensor("out", [rows, OUT_F], F32,
                            kind="ExternalOutput")
    with tile.TileContext(nc) as tc, ExitStack() as ctx:
        _body(ctx, tc, x_dram.ap(), w_dram.ap(), wql.ap(), wqa.ap(),
              o_dram.ap(), m_loc)
    nc.compile()
    _NC_CACHE[m_loc] = nc
    return nc


def run_sharded(x2d, weight, m_loc=M_LOC, trace=False):
    """x2d: [8192, 4096] f32 (full rows), weight: [4096, 4096] f32."""
    nc = build_nc(m_loc)
    rl = m_loc * P
    in_maps = []
    for c in range(N_CORES):
        wrows = [weight[((c + g) % N_CORES) * W_SHARD:
                        (((c + g) % N_CORES) + 1) * W_SHARD]
                 for g in range(N_LOC)]
        in_maps.append(
            {"x_in": np.ascontiguousarray(x2d[c * 1024:c * 1024 + rl]),
             "w_in": np.ascontiguousarray(np.concatenate(wrows, axis=0))})
    res = run_bass_kernel_spmd(nc, in_maps, core_ids=list(range(N_CORES)),
                               trace=trace)
    out = np.concatenate([res.results[c]["out"] for c in range(N_CORES)],
                         axis=0)
    rows = [c * 1024 + r for c in range(N_CORES) for r in range(rl)]
    return out, rows, res


def kernel(x, weight):
    b, s, f = x.shape
    x2d = np.ascontiguousarray(x.reshape(b * s, f)).astype(np.float32,
                                                           copy=False)
    w = np.ascontiguousarray(weight).astype(np.float32, copy=False)
    out, _, _ = run_sharded(x2d, w, M_LOC)
    return out.reshape(b, s, OUT_F).astype(np.float32, copy=False)
